# revision 1
# baseline (speedup 1.0000x reference)
"""Multi-head attention (lazy K/V projections) Trainium2 Bass kernel.

Problem: nn_MultiHeadAttention_54520314856024
  B=8, SQ=SK=1024, D=1024, E=128, H=32
  keys  = einsum('bsd,hde->hbse', states, Wk) + bk
  vals  = einsum('bsd,hde->hbse', states, Wv) + bv
  attn  = softmax(einsum('bqe,hbke->hbqk', query, keys) / sqrt(E))
  ctx   = einsum('hbqk,hbke->hbqe', attn, vals) -> concat heads -> @ Wc + bc

Sharding: batch-parallel, one batch element per NeuronCore (8 cores).
Each core runs the full H=32 head computation for its batch element;
outputs are stacked on the host. No collectives needed.

Core kernel layout (per core, everything transposed so contractions sit on
the partition axis):
  statesT [D, SK] and queryT [E, SQ] resident in SBUF.
  keysT_h [E, SK]   = Wk_h^T @ statesT          (PE, fp32r)
  vals    [SK, GE]  = statesT^T @ Wv_group      (PE, fp32r, G=4 heads/group)
  scoresT [SK, SQ]  = keysT^T  @ queryT         (PE, k on partitions)
  exp     = exp(scoresT / sqrt(E))              (ACT, no max-subtraction:
                                                 scores are O(1) by construction)
  denom_bcast [P, SQ] = ones^T @ exp            (PE partition-reduce+broadcast)
  ctx_rawT [E, SQ]  = vals^T @ exp              (PE)
  ctxT = ctx_rawT * 1/denom                     (DVE, reciprocal_approx_fast)
  finalT [E, SQ]   += Wc_h^T @ ctxT_h           (PE, accumulated over heads on DVE)
  out [SQ, E] = transpose(finalT) + bc          (PE transpose)
"""

import sys

for _p in ("/opt/trn_rl_repo",):
    if _p not in sys.path:
        sys.path.insert(0, _p)

import numpy as np

import concourse.bass as bass
import concourse.mybir as mybir
import concourse.tile as tile
from concourse import bacc, bass_utils
from concourse.masks import make_identity

B, SQ, SK = 8, 1024, 1024
D, E, H = 1024, 128, 32
P = 128          # partition width
DCH = D // P     # 8 d-chunks
KT = SK // P     # 8 k-tiles
G = 4            # heads per vals-group
NG = H // G      # 8 groups
NHALF = 512      # matmul moving-dim chunk (fp32 max)
SCALE = 1.0 / float(np.sqrt(E))

F32 = mybir.dt.float32
F32R = mybir.dt.float32r

N_CORES = 8

_COMPILED = {}
_ONES_SQ = np.ones((P, P), np.float32)
_ONES_R = np.ones((1, P), np.float32)


def build_nc(mm_dtype="f32r"):
    """Build the single-core Bass program (SPMD across 8 cores)."""
    MT = F32R if mm_dtype == "f32r" else F32

    nc = bacc.Bacc("TRN2", target_bir_lowering=False, debug=False)

    statesT = nc.dram_tensor("statesT", [D, SK], MT, kind="ExternalInput").ap()
    queryT = nc.dram_tensor("queryT", [E, SQ], MT, kind="ExternalInput").ap()
    WkT = nc.dram_tensor("WkT", [D, H * E], MT, kind="ExternalInput").ap()
    WvT = nc.dram_tensor("WvT", [D, H * E], MT, kind="ExternalInput").ap()
    Wc = nc.dram_tensor("Wc", [H * E, E], MT, kind="ExternalInput").ap()
    bkT = nc.dram_tensor("bkT", [E, H], F32, kind="ExternalInput").ap()
    bvF = nc.dram_tensor("bvF", [1, H * E], MT, kind="ExternalInput").ap()
    bcT = nc.dram_tensor("bcT", [E, 1], F32, kind="ExternalInput").ap()
    onesSQ = nc.dram_tensor("onesSQ", [P, P], MT, kind="ExternalInput").ap()
    onesR = nc.dram_tensor("onesR", [1, P], MT, kind="ExternalInput").ap()
    out = nc.dram_tensor("out", [SQ, E], F32, kind="ExternalOutput").ap()

    Wc3 = Wc.rearrange("(h e) f -> h e f", e=P)

    from contextlib import ExitStack

    with tile.TileContext(nc) as tc, ExitStack() as es:
        if True:
            constp = es.enter_context(tc.tile_pool(name="const", bufs=1))
            statesp = es.enter_context(tc.tile_pool(name="states", bufs=DCH))
            queryp = es.enter_context(tc.tile_pool(name="query", bufs=1))
            wkp = es.enter_context(tc.tile_pool(name="wk", bufs=12))
            wvp = es.enter_context(tc.tile_pool(name="wv", bufs=16))
            wcp = es.enter_context(tc.tile_pool(name="wc", bufs=4))
            keysp = es.enter_context(tc.tile_pool(name="keys", bufs=2))
            expp = es.enter_context(tc.tile_pool(name="exps", bufs=8))
            valsp = es.enter_context(tc.tile_pool(name="vals", bufs=12))
            recipp = es.enter_context(tc.tile_pool(name="recip", bufs=2))
            ctxp = es.enter_context(tc.tile_pool(name="ctx", bufs=2))
            finalp = es.enter_context(tc.tile_pool(name="final", bufs=1))
            outp = es.enter_context(tc.tile_pool(name="outs", bufs=KT))
            ps_score = es.enter_context(tc.tile_pool(name="ps_score", bufs=1, space="PSUM"))
            ps_denom = es.enter_context(tc.tile_pool(name="ps_denom", bufs=1, space="PSUM"))
            ps_ctx = es.enter_context(tc.tile_pool(name="ps_ctx", bufs=1, space="PSUM"))
            ps_kvf = es.enter_context(tc.tile_pool(name="ps_kvf", bufs=2, space="PSUM"))
            # ---- constants ----
            ones_sq = constp.tile([P, P], MT)
            nc.sync.dma_start(ones_sq[:], onesSQ[:])
            ones_row = constp.tile([1, P], MT)
            nc.sync.dma_start(ones_row[:], onesR[:])
            ident = constp.tile([P, P], F32)
            make_identity(nc, ident[:])
            bk_t = constp.tile([E, H], F32)
            nc.sync.dma_start(bk_t[:], bkT[:])
            bv_t = constp.tile([1, H * E], MT)
            nc.sync.dma_start(bv_t[:], bvF[:])
            bc_t = constp.tile([E, 1], F32)
            nc.sync.dma_start(bc_t[:], bcT[:])

            # ---- resident activations ----
            st = []
            for d in range(DCH):
                st_t = statesp.tile([P, SK], MT)
                nc.sync.dma_start(st_t[:], statesT[d * P : (d + 1) * P, :])
                st.append(st_t)
            q_t = queryp.tile([E, SQ], MT)
            nc.sync.dma_start(q_t[:], queryT[:])

            final_t = finalp.tile([E, SQ], F32)

            for g in range(NG):
                # ---- vals for this head-group: vals[k, (g4,e)] ----
                wv_tiles = []
                for d in range(DCH):
                    wv_t = wvp.tile([P, G * E], MT)
                    nc.sync.dma_start(
                        wv_t[:],
                        WvT[d * P : (d + 1) * P, g * G * E : (g + 1) * G * E],
                    )
                    wv_tiles.append(wv_t)
                wk_tiles = []
                for d in range(DCH):
                    wk_t = wkp.tile([P, G * E], MT)
                    nc.sync.dma_start(
                        wk_t[:],
                        WkT[d * P : (d + 1) * P, g * G * E : (g + 1) * G * E],
                    )
                    wk_tiles.append(wk_t)

                vals_tiles = []
                for kt in range(KT):
                    pv = ps_kvf.tile([P, G * E], F32, tag="kvf")
                    for d in range(DCH):
                        nc.tensor.matmul(
                            pv[:],
                            (st[d][:, kt * P : (kt + 1) * P]),
                            (wv_tiles[d][:]),
                            start=(d == 0),
                            stop=False,
                        )
                    # bias: vals[k, he] += bv[he] (rank-1 accumulate)
                    nc.tensor.matmul(
                        pv[:],
                        (ones_row[:]),
                        (bv_t[:, g * G * E : (g + 1) * G * E]),
                        start=False,
                        stop=True,
                    )
                    v_sb = valsp.tile([P, G * E], MT)
                    nc.vector.tensor_copy(v_sb[:], pv[:])
                    vals_tiles.append(v_sb)

                for hg in range(G):
                    h = g * G + hg
                    # ---- keysT: [E, SK] ----
                    keys_sb = keysp.tile([E, SK], MT)
                    for half in range(2):
                        pk = ps_kvf.tile([P, NHALF], F32, tag="kvf")
                        for d in range(DCH):
                            nc.tensor.matmul(
                                pk[:],
                                (wk_tiles[d][:, hg * E : (hg + 1) * E]),
                                (st[d][:, half * NHALF : (half + 1) * NHALF]),
                                start=(d == 0),
                                stop=(d == DCH - 1),
                            )
                        nc.vector.tensor_scalar(
                            keys_sb[:, half * NHALF : (half + 1) * NHALF],
                            pk[:],
                            bk_t[:, h : h + 1],
                            None,
                            op0=mybir.AluOpType.add,
                        )

                    # ---- scoresT + exp: [SK, SQ] by k-tile ----
                    exp_tiles = []
                    for kt in range(KT):
                        ps = ps_score.tile([P, SQ], F32, tag="score")
                        for qh in range(2):
                            nc.tensor.matmul(
                                ps[:, qh * NHALF : (qh + 1) * NHALF],
                                (keys_sb[:, kt * P : (kt + 1) * P]),
                                (q_t[:, qh * NHALF : (qh + 1) * NHALF]),
                                start=True,
                                stop=True,
                            )
                        ex = expp.tile([P, SQ], MT)
                        nc.scalar.activation(
                            ex[:], ps[:], mybir.ActivationFunctionType.Exp,
                            scale=SCALE,
                        )
                        exp_tiles.append(ex)

                    # ---- denominator, reduced over k and broadcast on PE ----
                    pd = ps_denom.tile([P, SQ], F32, tag="denom")
                    for kt in range(KT):
                        for qh in range(2):
                            nc.tensor.matmul(
                                pd[:, qh * NHALF : (qh + 1) * NHALF],
                                (ones_sq[:]),
                                (exp_tiles[kt][:, qh * NHALF : (qh + 1) * NHALF]),
                                start=(kt == 0),
                                stop=(kt == KT - 1),
                            )
                    rec = recipp.tile([P, SQ], F32)
                    nc.vector.reciprocal_approx_fast(out=rec[:], in_=pd[:])

                    # ---- ctx_rawT: [E, SQ] ----
                    pc = ps_ctx.tile([E, SQ], F32, tag="ctx")
                    for kt in range(KT):
                        for qh in range(2):
                            nc.tensor.matmul(
                                pc[:, qh * NHALF : (qh + 1) * NHALF],
                                (vals_tiles[kt][:, hg * E : (hg + 1) * E]),
                                (exp_tiles[kt][:, qh * NHALF : (qh + 1) * NHALF]),
                                start=(kt == 0),
                                stop=(kt == KT - 1),
                            )
                    ctx_sb = ctxp.tile([E, SQ], MT)
                    nc.vector.tensor_mul(ctx_sb[:], pc[:], rec[:])

                    # ---- final projection contribution ----
                    wc_t = wcp.tile([P, P], MT)
                    nc.sync.dma_start(wc_t[:], Wc3[h])
                    for qh in range(2):
                        pf = ps_kvf.tile([P, NHALF], F32, tag="kvf")
                        nc.tensor.matmul(
                            pf[:],
                            (wc_t[:]),
                            (ctx_sb[:, qh * NHALF : (qh + 1) * NHALF]),
                            start=True,
                            stop=True,
                        )
                        if h == 0:
                            nc.vector.tensor_scalar(
                                final_t[:, qh * NHALF : (qh + 1) * NHALF],
                                pf[:],
                                bc_t[:],
                                None,
                                op0=mybir.AluOpType.add,
                            )
                        else:
                            nc.vector.tensor_add(
                                final_t[:, qh * NHALF : (qh + 1) * NHALF],
                                final_t[:, qh * NHALF : (qh + 1) * NHALF],
                                pf[:],
                            )

            # ---- transpose finalT -> out [SQ, E] ----
            for qt in range(KT):
                pt = ps_kvf.tile([P, P], F32, tag="kvf")
                nc.tensor.transpose(
                    pt[:], final_t[:, qt * P : (qt + 1) * P], ident[:]
                )
                o_sb = outp.tile([P, E], F32)
                nc.vector.tensor_copy(o_sb[:], pt[:])
                nc.sync.dma_start(out[qt * P : (qt + 1) * P, :], o_sb[:])

    nc.compile()
    return nc


def _prep_inputs(query, states, Wk, bk, Wv, bv, Wc, bc):
    """Host-side sharding: per-core input maps (core c == batch element c)."""
    query = np.asarray(query, np.float32)
    states = np.asarray(states, np.float32)
    Wk = np.asarray(Wk, np.float32)
    bk = np.asarray(bk, np.float32)
    Wv = np.asarray(Wv, np.float32)
    bv = np.asarray(bv, np.float32)
    Wc = np.asarray(Wc, np.float32)
    bc = np.asarray(bc, np.float32)

    WkT = np.ascontiguousarray(Wk.transpose(1, 0, 2).reshape(D, H * E))
    WvT = np.ascontiguousarray(Wv.transpose(1, 0, 2).reshape(D, H * E))
    bkT = np.ascontiguousarray(bk.T)                      # [E, H]
    bvF = np.ascontiguousarray(bv.reshape(1, H * E))
    bcT = np.ascontiguousarray(bc.reshape(E, 1))
    WcC = np.ascontiguousarray(Wc)

    in_maps = []
    for c in range(N_CORES):
        in_maps.append(
            {
                "statesT": np.ascontiguousarray(states[c].T),  # [D, SK]
                "queryT": np.ascontiguousarray(query[c].T),    # [E, SQ]
                "WkT": WkT,
                "WvT": WvT,
                "Wc": WcC,
                "bkT": bkT,
                "bvF": bvF,
                "bcT": bcT,
                "onesSQ": _ONES_SQ,
                "onesR": _ONES_R,
            }
        )
    return in_maps


def get_nc(mm_dtype="f32r"):
    nc = _COMPILED.get(mm_dtype)
    if nc is None:
        nc = build_nc(mm_dtype)
        _COMPILED[mm_dtype] = nc
    return nc


def kernel(query, states, Wk, bk, Wv, bv, Wc, bc):
    nc = get_nc()
    in_maps = _prep_inputs(query, states, Wk, bk, Wv, bv, Wc, bc)
    res = bass_utils.run_bass_kernel_spmd(nc, in_maps, list(range(N_CORES)))
    return np.stack([res.results[c]["out"] for c in range(N_CORES)], axis=0)



# revision 2
# speedup vs baseline: 1.4005x; 1.4005x over previous
"""Multi-head attention (lazy K/V projections) Trainium2 Bass kernel, v2.

Problem: nn_MultiHeadAttention_54520314856024
  B=8, SQ=SK=1024, D=1024, E=128, H=32
  keys  = einsum('bsd,hde->hbse', states, Wk) + bk
  vals  = einsum('bsd,hde->hbse', states, Wv) + bv
  attn  = softmax(einsum('bqe,hbke->hbqk', query, keys) / sqrt(E))
  ctx   = einsum('hbqk,hbke->hbqe', attn, vals) -> concat heads -> @ Wc + bc

Sharding: batch-parallel, one batch element per NeuronCore (8 cores).

v2 changes vs v1:
  - bk dropped on device: softmax over k is invariant to the per-q additive
    shift (bk . q), so keys bias cancels exactly.
  - bv folded into bc on host: sum_k attn = 1 implies ctx = ctx0 + bv, so
    out = ctx0 @ Wc + (bc + bv.flatten() @ Wc). Removes all vals-bias matmuls.
  - scores PSUM split into [128,512] single-bank tiles with a 2-buf pool so
    scores(kt+1) overlaps exp(kt); exp runs per 512-wide half.
  - denominator/ctx accumulate per 512-half in dedicated single-bank tiles.
"""

import sys

for _p in ("/opt/trn_rl_repo",):
    if _p not in sys.path:
        sys.path.insert(0, _p)

import numpy as np

import concourse.bass as bass
import concourse.mybir as mybir
import concourse.tile as tile
from concourse import bacc, bass_utils
from concourse.masks import make_identity

B, SQ, SK = 8, 1024, 1024
D, E, H = 1024, 128, 32
P = 128          # partition width
DCH = D // P     # 8 d-chunks
KT = SK // P     # 8 k-tiles
G = 4            # heads per vals-group
NG = H // G      # 8 groups
NHALF = 512      # matmul moving-dim chunk (fp32 max)
SCALE = 1.0 / float(np.sqrt(E))

F32 = mybir.dt.float32
F32R = mybir.dt.float32r

N_CORES = 8

_COMPILED = {}
_ONES_SQ = np.ones((P, P), np.float32)


def build_nc(mm_dtype="f32r"):
    """Build the single-core Bass program (SPMD across 8 cores)."""
    MT = F32R if mm_dtype == "f32r" else F32

    nc = bacc.Bacc("TRN2", target_bir_lowering=False, debug=False)

    statesT = nc.dram_tensor("statesT", [D, SK], MT, kind="ExternalInput").ap()
    queryT = nc.dram_tensor("queryT", [E, SQ], MT, kind="ExternalInput").ap()
    WkT = nc.dram_tensor("WkT", [D, H * E], MT, kind="ExternalInput").ap()
    WvT = nc.dram_tensor("WvT", [D, H * E], MT, kind="ExternalInput").ap()
    Wc = nc.dram_tensor("Wc", [H * E, E], MT, kind="ExternalInput").ap()
    bcT = nc.dram_tensor("bcT", [E, 1], F32, kind="ExternalInput").ap()
    onesSQ = nc.dram_tensor("onesSQ", [P, P], MT, kind="ExternalInput").ap()
    out = nc.dram_tensor("out", [SQ, E], F32, kind="ExternalOutput").ap()

    Wc3 = Wc.rearrange("(h e) f -> h e f", e=P)

    from contextlib import ExitStack

    with tile.TileContext(nc) as tc, ExitStack() as es:
        constp = es.enter_context(tc.tile_pool(name="const", bufs=1))
        statesp = es.enter_context(tc.tile_pool(name="states", bufs=DCH))
        queryp = es.enter_context(tc.tile_pool(name="query", bufs=1))
        wkp = es.enter_context(tc.tile_pool(name="wk", bufs=10))
        wvp = es.enter_context(tc.tile_pool(name="wv", bufs=10))
        wcp = es.enter_context(tc.tile_pool(name="wc", bufs=4))
        keysp = es.enter_context(tc.tile_pool(name="keys", bufs=2))
        expp = es.enter_context(tc.tile_pool(name="exps", bufs=8))
        valsp = es.enter_context(tc.tile_pool(name="vals", bufs=12))
        recipp = es.enter_context(tc.tile_pool(name="recip", bufs=2))
        ctxp = es.enter_context(tc.tile_pool(name="ctx", bufs=2))
        finalp = es.enter_context(tc.tile_pool(name="final", bufs=1))
        outp = es.enter_context(tc.tile_pool(name="outs", bufs=4))
        ps_main = es.enter_context(tc.tile_pool(name="ps_main", bufs=3, space="PSUM"))
        ps_denom = es.enter_context(tc.tile_pool(name="ps_denom", bufs=2, space="PSUM"))
        ps_ctx = es.enter_context(tc.tile_pool(name="ps_ctx", bufs=2, space="PSUM"))

        # ---- constants ----
        ones_sq = constp.tile([P, P], MT)
        nc.sync.dma_start(ones_sq[:], onesSQ[:])
        ident = constp.tile([P, P], F32)
        make_identity(nc, ident[:])
        bc_t = constp.tile([E, 1], F32)
        nc.sync.dma_start(bc_t[:], bcT[:])

        # ---- resident activations ----
        st = []
        for d in range(DCH):
            st_t = statesp.tile([P, SK], MT)
            nc.sync.dma_start(st_t[:], statesT[d * P : (d + 1) * P, :])
            st.append(st_t)
        q_t = queryp.tile([E, SQ], MT)
        nc.sync.dma_start(q_t[:], queryT[:])

        final_t = finalp.tile([E, SQ], F32)

        def emit_final(h, ctx_sb):
            """Final projection contribution of head h into final_t."""
            wc_t = wcp.tile([P, P], MT, name=f"wc_t{h}")
            nc.sync.dma_start(wc_t[:], Wc3[h])
            for qh in range(2):
                pf = ps_denom.tile([P, NHALF], F32, tag="denom", name=f"pf{h}_{qh}")
                nc.tensor.matmul(
                    pf[:],
                    (wc_t[:]),
                    (ctx_sb[:, qh * NHALF : (qh + 1) * NHALF]),
                    start=True,
                    stop=True,
                )
                if h == 0:
                    nc.vector.tensor_scalar(
                        final_t[:, qh * NHALF : (qh + 1) * NHALF],
                        pf[:],
                        bc_t[:],
                        None,
                        op0=mybir.AluOpType.add,
                    )
                else:
                    nc.vector.tensor_add(
                        final_t[:, qh * NHALF : (qh + 1) * NHALF],
                        final_t[:, qh * NHALF : (qh + 1) * NHALF],
                        pf[:],
                    )

        pending_final = None

        for g in range(NG):
            # ---- vals for this head-group: vals[k, (g4,e)] ----
            wv_tiles = []
            for d in range(DCH):
                wv_t = wvp.tile([P, G * E], MT)
                nc.sync.dma_start(
                    wv_t[:],
                    WvT[d * P : (d + 1) * P, g * G * E : (g + 1) * G * E],
                )
                wv_tiles.append(wv_t)
            wk_tiles = []
            for d in range(DCH):
                wk_t = wkp.tile([P, G * E], MT)
                nc.sync.dma_start(
                    wk_t[:],
                    WkT[d * P : (d + 1) * P, g * G * E : (g + 1) * G * E],
                )
                wk_tiles.append(wk_t)

            vals_tiles = []
            for kt in range(KT):
                pv = ps_main.tile([P, G * E], F32, tag="m")
                for d in range(DCH):
                    nc.tensor.matmul(
                        pv[:],
                        (st[d][:, kt * P : (kt + 1) * P]),
                        (wv_tiles[d][:]),
                        start=(d == 0),
                        stop=(d == DCH - 1),
                    )
                v_sb = valsp.tile([P, G * E], MT)
                nc.vector.tensor_copy(v_sb[:], pv[:])
                vals_tiles.append(v_sb)

            for hg in range(G):
                h = g * G + hg
                # ---- keysT: [E, SK] (no bias: softmax-shift invariant) ----
                keys_sb = keysp.tile([E, SK], MT)
                for half in range(2):
                    pk = ps_main.tile([P, NHALF], F32, tag="m")
                    for d in range(DCH):
                        nc.tensor.matmul(
                            pk[:],
                            (wk_tiles[d][:, hg * E : (hg + 1) * E]),
                            (st[d][:, half * NHALF : (half + 1) * NHALF]),
                            start=(d == 0),
                            stop=(d == DCH - 1),
                        )
                    nc.vector.tensor_copy(
                        keys_sb[:, half * NHALF : (half + 1) * NHALF], pk[:]
                    )

                # previous head's final projection: PE runs it after keys,
                # by which time DVE recip/mul of head h-1 has completed.
                if pending_final is not None:
                    emit_final(*pending_final)
                    pending_final = None

                # ---- scores -> exp -> denom/ctx accumulate, per (kt, qh) ----
                pd = [
                    ps_denom.tile([P, NHALF], F32, tag="denom", name=f"pd{h}_{i}")
                    for i in range(2)
                ]
                pc = [
                    ps_ctx.tile([P, NHALF], F32, tag="ctx", name=f"pc{h}_{i}")
                    for i in range(2)
                ]
                # Software-pipelined: emit kt's scores+exp, then kt-1's
                # denom/ctx (so PE never waits on the just-issued exp).
                pending = None  # (kt, [ex_qh0, ex_qh1])
                for kt in range(KT):
                    exs = []
                    for qh in range(2):
                        ps = ps_main.tile([P, NHALF], F32, tag="m")
                        nc.tensor.matmul(
                            ps[:],
                            (keys_sb[:, kt * P : (kt + 1) * P]),
                            (q_t[:, qh * NHALF : (qh + 1) * NHALF]),
                            start=True,
                            stop=True,
                        )
                        ex = expp.tile([P, NHALF], MT)
                        nc.scalar.activation(
                            ex[:], ps[:], mybir.ActivationFunctionType.Exp,
                            scale=SCALE,
                        )
                        exs.append(ex)
                    if pending is not None:
                        pkt, pexs = pending
                        for qh in range(2):
                            nc.tensor.matmul(
                                pd[qh][:],
                                (ones_sq[:]),
                                (pexs[qh][:]),
                                start=(pkt == 0),
                                stop=False,
                            )
                            nc.tensor.matmul(
                                pc[qh][:],
                                (vals_tiles[pkt][:, hg * E : (hg + 1) * E]),
                                (pexs[qh][:]),
                                start=(pkt == 0),
                                stop=False,
                            )
                    pending = (kt, exs)
                pkt, pexs = pending
                for qh in range(2):
                    nc.tensor.matmul(
                        pd[qh][:],
                        (ones_sq[:]),
                        (pexs[qh][:]),
                        start=False,
                        stop=True,
                    )
                    nc.tensor.matmul(
                        pc[qh][:],
                        (vals_tiles[pkt][:, hg * E : (hg + 1) * E]),
                        (pexs[qh][:]),
                        start=False,
                        stop=True,
                    )

                # ---- normalize: ctx_sb = pc / denom ----
                ctx_sb = ctxp.tile([E, SQ], MT)
                for qh in range(2):
                    rec = recipp.tile([P, NHALF], F32)
                    nc.vector.reciprocal_approx_fast(out=rec[:], in_=pd[qh][:])
                    nc.vector.tensor_mul(
                        ctx_sb[:, qh * NHALF : (qh + 1) * NHALF], pc[qh][:], rec[:]
                    )

                # defer the final projection until after the next head's keys
                pending_final = (h, ctx_sb)

        emit_final(*pending_final)
        pending_final = None

        # ---- transpose finalT -> out [SQ, E] ----
        for qt in range(KT):
            pt = ps_main.tile([P, P], F32, tag="m")
            nc.tensor.transpose(
                pt[:], final_t[:, qt * P : (qt + 1) * P], ident[:]
            )
            o_sb = outp.tile([P, E], F32)
            nc.vector.tensor_copy(o_sb[:], pt[:])
            nc.sync.dma_start(out[qt * P : (qt + 1) * P, :], o_sb[:])

    nc.compile()
    return nc


def _prep_inputs(query, states, Wk, bk, Wv, bv, Wc, bc):
    """Host-side sharding: per-core input maps (core c == batch element c).

    bk is dropped (softmax shift invariance); bv is folded into bc:
    out = ctx0 @ Wc + (bc + bv.flatten() @ Wc).
    """
    query = np.asarray(query, np.float32)
    states = np.asarray(states, np.float32)
    Wk = np.asarray(Wk, np.float32)
    Wv = np.asarray(Wv, np.float32)
    Wc = np.asarray(Wc, np.float32)
    bv = np.asarray(bv, np.float32)
    bc = np.asarray(bc, np.float32)

    WkT = np.ascontiguousarray(Wk.transpose(1, 0, 2).reshape(D, H * E))
    WvT = np.ascontiguousarray(Wv.transpose(1, 0, 2).reshape(D, H * E))
    bc_eff = (
        np.asarray(bc, np.float64) + np.asarray(bv, np.float64).reshape(H * E) @ np.asarray(Wc, np.float64)
    ).astype(np.float32)
    bcT = np.ascontiguousarray(bc_eff.reshape(E, 1))
    WcC = np.ascontiguousarray(Wc)

    in_maps = []
    for c in range(N_CORES):
        in_maps.append(
            {
                "statesT": np.ascontiguousarray(states[c].T),  # [D, SK]
                "queryT": np.ascontiguousarray(query[c].T),    # [E, SQ]
                "WkT": WkT,
                "WvT": WvT,
                "Wc": WcC,
                "bcT": bcT,
                "onesSQ": _ONES_SQ,
            }
        )
    return in_maps


def get_nc(mm_dtype="f32r"):
    nc = _COMPILED.get(mm_dtype)
    if nc is None:
        nc = build_nc(mm_dtype)
        _COMPILED[mm_dtype] = nc
    return nc


def kernel(query, states, Wk, bk, Wv, bv, Wc, bc):
    nc = get_nc()
    in_maps = _prep_inputs(query, states, Wk, bk, Wv, bv, Wc, bc)
    res = bass_utils.run_bass_kernel_spmd(nc, in_maps, list(range(N_CORES)))
    return np.stack([res.results[c]["out"] for c in range(N_CORES)], axis=0)


# revision 3
# speedup vs baseline: 1.7705x; 1.2642x over previous
"""Multi-head attention (lazy K/V projections) Trainium2 Bass kernel.

Problem: nn_MultiHeadAttention_54520314856024
  B=8, SQ=SK=1024, D=1024, E=128, H=32
  keys  = einsum('bsd,hde->hbse', states, Wk) + bk
  vals  = einsum('bsd,hde->hbse', states, Wv) + bv
  attn  = softmax(einsum('bqe,hbke->hbqk', query, keys) / sqrt(E))
  ctx   = einsum('hbqk,hbke->hbqe', attn, vals) -> concat heads -> @ Wc + bc

Sharding: batch-parallel, one batch element per NeuronCore (8 cores).

Design notes:
  - bk dropped on device: softmax over k is invariant to the per-q additive
    shift (bk . q), so the keys bias cancels exactly.
  - bv folded into bc on host: sum_k attn = 1 implies ctx = ctx0 + bv, so
    out = ctx0 @ Wc + (bc + bv.flatten() @ Wc). Removes all vals-bias work.
  - scores/keys/vals/transposes share one 3-buffer single-bank PSUM pool
    (their phases are disjoint); denominator + final-projection share a
    2-buffer pool; ctx accumulators a 2-buffer pool. 7 PSUM banks total.
  - per head, the kt loop emits scores+exp for kt and denominator/ctx
    matmuls for kt-1 (software pipelining) so PE never waits on ACT's exp.
  - the final projection of head h is deferred until after head h+1's keys
    so DVE recip/mul latency hides under PE keys matmuls.
"""

import sys

for _p in ("/opt/trn_rl_repo",):
    if _p not in sys.path:
        sys.path.insert(0, _p)

import numpy as np

import concourse.bass as bass
import concourse.mybir as mybir
import concourse.tile as tile
from concourse import bacc, bass_utils
from concourse.masks import make_identity

B, SQ, SK = 8, 1024, 1024
D, E, H = 1024, 128, 32
P = 128          # partition width
DCH = D // P     # 8 d-chunks
KT = SK // P     # 8 k-tiles
G = 4            # heads per vals-group
NG = H // G      # 8 groups
NHALF = 512      # matmul moving-dim chunk (fp32 max)
SCALE = 1.0 / float(np.sqrt(E))

F32 = mybir.dt.float32
F32R = mybir.dt.float32r

N_CORES = 8

_COMPILED = {}
_ONES_SQ = np.ones((P, P), np.float32)


def build_nc(mm_dtype="f32r", repeat=1):
    """Build the single-core Bass program (SPMD across 8 cores).

    repeat > 1 re-emits the whole computation that many times (identical
    work each pass) for launch-overhead-amortized timing; the final DRAM
    output is written by every pass (all identical).
    """
    MT = F32R if mm_dtype == "f32r" else F32

    nc = bacc.Bacc("TRN2", target_bir_lowering=False, debug=False)

    statesT = nc.dram_tensor("statesT", [D, SK], MT, kind="ExternalInput").ap()
    queryT = nc.dram_tensor("queryT", [E, SQ], MT, kind="ExternalInput").ap()
    WkT = nc.dram_tensor("WkT", [D, H * E], MT, kind="ExternalInput").ap()
    WvT = nc.dram_tensor("WvT", [D, H * E], MT, kind="ExternalInput").ap()
    Wc = nc.dram_tensor("Wc", [H * E, E], MT, kind="ExternalInput").ap()
    bcT = nc.dram_tensor("bcT", [E, 1], F32, kind="ExternalInput").ap()
    onesSQ = nc.dram_tensor("onesSQ", [P, P], MT, kind="ExternalInput").ap()
    out = nc.dram_tensor("out", [SQ, E], F32, kind="ExternalOutput").ap()

    Wc3 = Wc.rearrange("(h e) f -> h e f", e=P)

    from contextlib import ExitStack

    with tile.TileContext(nc) as tc, ExitStack() as es:
        constp = es.enter_context(tc.tile_pool(name="const", bufs=1))
        statesp = es.enter_context(tc.tile_pool(name="states", bufs=DCH))
        queryp = es.enter_context(tc.tile_pool(name="query", bufs=1))
        wkp = es.enter_context(tc.tile_pool(name="wk", bufs=10))
        wvp = es.enter_context(tc.tile_pool(name="wv", bufs=10))
        wcp = es.enter_context(tc.tile_pool(name="wc", bufs=4))
        keysp = es.enter_context(tc.tile_pool(name="keys", bufs=2))
        expp = es.enter_context(tc.tile_pool(name="exps", bufs=8))
        valsp = es.enter_context(tc.tile_pool(name="vals", bufs=12))
        recipp = es.enter_context(tc.tile_pool(name="recip", bufs=2))
        ctxp = es.enter_context(tc.tile_pool(name="ctx", bufs=2))
        finalp = es.enter_context(tc.tile_pool(name="final", bufs=1))
        outp = es.enter_context(tc.tile_pool(name="outs", bufs=4))
        ps_main = es.enter_context(tc.tile_pool(name="ps_main", bufs=3, space="PSUM"))
        ps_denom = es.enter_context(tc.tile_pool(name="ps_denom", bufs=2, space="PSUM"))
        ps_ctx = es.enter_context(tc.tile_pool(name="ps_ctx", bufs=2, space="PSUM"))

        # ---- constants ----
        ones_sq = constp.tile([P, P], MT)
        nc.sync.dma_start(ones_sq[:], onesSQ[:])
        ident = constp.tile([P, P], F32)
        make_identity(nc, ident[:])
        bc_t = constp.tile([E, 1], F32)
        nc.sync.dma_start(bc_t[:], bcT[:])

        # ---- resident activations ----
        st = []
        for d in range(DCH):
            st_t = statesp.tile([P, SK], MT, name="st_t")
            nc.sync.dma_start(st_t[:], statesT[d * P : (d + 1) * P, :])
            st.append(st_t)
        q_t = queryp.tile([E, SQ], MT)
        nc.sync.dma_start(q_t[:], queryT[:])

        final_t = finalp.tile([E, SQ], F32)

        def emit_final(rep, h, ctx_sb):
            """Final projection contribution of head h into final_t."""
            wc_t = wcp.tile([P, P], MT, name="wc_t")
            nc.sync.dma_start(wc_t[:], Wc3[h])
            for qh in range(2):
                pf = ps_denom.tile(
                    [P, NHALF], F32, tag="denom", name="pf"
                )
                nc.tensor.matmul(
                    pf[:],
                    (wc_t[:]),
                    (ctx_sb[:, qh * NHALF : (qh + 1) * NHALF]),
                    start=True,
                    stop=True,
                )
                if h == 0:
                    nc.vector.tensor_scalar(
                        final_t[:, qh * NHALF : (qh + 1) * NHALF],
                        pf[:],
                        bc_t[:],
                        None,
                        op0=mybir.AluOpType.add,
                    )
                else:
                    nc.vector.tensor_add(
                        final_t[:, qh * NHALF : (qh + 1) * NHALF],
                        final_t[:, qh * NHALF : (qh + 1) * NHALF],
                        pf[:],
                    )

        pending_final = None

        for rep in range(repeat):
            for g in range(NG):
                # ---- vals for this head-group: vals[k, (g4,e)] ----
                wv_tiles = []
                for d in range(DCH):
                    wv_t = wvp.tile([P, G * E], MT, name="wv_t")
                    nc.sync.dma_start(
                        wv_t[:],
                        WvT[d * P : (d + 1) * P, g * G * E : (g + 1) * G * E],
                    )
                    wv_tiles.append(wv_t)
                wk_tiles = []
                for d in range(DCH):
                    wk_t = wkp.tile([P, G * E], MT, name="wk_t")
                    nc.sync.dma_start(
                        wk_t[:],
                        WkT[d * P : (d + 1) * P, g * G * E : (g + 1) * G * E],
                    )
                    wk_tiles.append(wk_t)

                vals_tiles = []
                for kt in range(KT):
                    pv = ps_main.tile(
                        [P, G * E], F32, tag="m", name="pv"
                    )
                    for d in range(DCH):
                        nc.tensor.matmul(
                            pv[:],
                            (st[d][:, kt * P : (kt + 1) * P]),
                            (wv_tiles[d][:]),
                            start=(d == 0),
                            stop=(d == DCH - 1),
                        )
                    v_sb = valsp.tile([P, G * E], MT, name="v_sb")
                    nc.vector.tensor_copy(v_sb[:], pv[:])
                    vals_tiles.append(v_sb)

                for hg in range(G):
                    h = g * G + hg
                    # ---- keysT: [E, SK] (no bias: softmax-shift invariant) ----
                    keys_sb = keysp.tile([E, SK], MT, name="keys_sb")
                    for half in range(2):
                        pk = ps_main.tile(
                            [P, NHALF], F32, tag="m", name="pk"
                        )
                        for d in range(DCH):
                            nc.tensor.matmul(
                                pk[:],
                                (wk_tiles[d][:, hg * E : (hg + 1) * E]),
                                (st[d][:, half * NHALF : (half + 1) * NHALF]),
                                start=(d == 0),
                                stop=(d == DCH - 1),
                            )
                        nc.vector.tensor_copy(
                            keys_sb[:, half * NHALF : (half + 1) * NHALF], pk[:]
                        )

                    # previous head's final projection: PE runs it after keys,
                    # by which time DVE recip/mul of head h-1 has completed.
                    if pending_final is not None:
                        emit_final(*pending_final)
                        pending_final = None

                    # ---- scores -> exp -> denom/ctx accumulate, per (kt, qh) ----
                    pd = [
                        ps_denom.tile(
                            [P, NHALF], F32, tag="denom", name="pd"
                        )
                        for i in range(2)
                    ]
                    pc = [
                        ps_ctx.tile(
                            [P, NHALF], F32, tag="ctx", name="pc"
                        )
                        for i in range(2)
                    ]
                    # Software-pipelined: emit kt's scores+exp, then kt-1's
                    # denom/ctx (so PE never waits on the just-issued exp).
                    pending = None  # ([ex_qh0, ex_qh1], kt)
                    for kt in range(KT):
                        exs = []
                        for qh in range(2):
                            ps = ps_main.tile(
                                [P, NHALF], F32, tag="m",
                                name="ps",
                            )
                            nc.tensor.matmul(
                                ps[:],
                                (keys_sb[:, kt * P : (kt + 1) * P]),
                                (q_t[:, qh * NHALF : (qh + 1) * NHALF]),
                                start=True,
                                stop=True,
                            )
                            ex = expp.tile(
                                [P, NHALF], MT, name="ex"
                            )
                            nc.scalar.activation(
                                ex[:], ps[:], mybir.ActivationFunctionType.Exp,
                                scale=SCALE,
                            )
                            exs.append(ex)
                        if pending is not None:
                            pexs, pkt = pending
                            for qh in range(2):
                                nc.tensor.matmul(
                                    pd[qh][:],
                                    (ones_sq[:]),
                                    (pexs[qh][:]),
                                    start=(pkt == 0),
                                    stop=False,
                                )
                                nc.tensor.matmul(
                                    pc[qh][:],
                                    (vals_tiles[pkt][:, hg * E : (hg + 1) * E]),
                                    (pexs[qh][:]),
                                    start=(pkt == 0),
                                    stop=False,
                                )
                        pending = (exs, kt)
                    pexs, pkt = pending
                    for qh in range(2):
                        nc.tensor.matmul(
                            pd[qh][:],
                            (ones_sq[:]),
                            (pexs[qh][:]),
                            start=False,
                            stop=True,
                        )
                        nc.tensor.matmul(
                            pc[qh][:],
                            (vals_tiles[pkt][:, hg * E : (hg + 1) * E]),
                            (pexs[qh][:]),
                            start=False,
                            stop=True,
                        )

                    # ---- normalize: ctx_sb = pc / denom ----
                    ctx_sb = ctxp.tile([E, SQ], MT, name="ctx_sb")
                    for qh in range(2):
                        rec = recipp.tile(
                            [P, NHALF], F32, name="rec"
                        )
                        nc.vector.reciprocal_approx_fast(out=rec[:], in_=pd[qh][:])
                        nc.vector.tensor_mul(
                            ctx_sb[:, qh * NHALF : (qh + 1) * NHALF],
                            pc[qh][:],
                            rec[:],
                        )

                    # defer the final projection until after the next head's keys
                    pending_final = (rep, h, ctx_sb)

            emit_final(*pending_final)
            pending_final = None

            # ---- transpose finalT -> out [SQ, E] ----
            for qt in range(KT):
                pt = ps_main.tile([P, P], F32, tag="m", name="pt")
                nc.tensor.transpose(
                    pt[:], final_t[:, qt * P : (qt + 1) * P], ident[:]
                )
                o_sb = outp.tile([P, E], F32, name="o_sb")
                nc.vector.tensor_copy(o_sb[:], pt[:])
                nc.sync.dma_start(out[qt * P : (qt + 1) * P, :], o_sb[:])

    nc.compile()
    return nc


def _prep_inputs(query, states, Wk, bk, Wv, bv, Wc, bc):
    """Host-side sharding: per-core input maps (core c == batch element c).

    bk is dropped (softmax shift invariance); bv is folded into bc:
    out = ctx0 @ Wc + (bc + bv.flatten() @ Wc).
    """
    query = np.asarray(query, np.float32)
    states = np.asarray(states, np.float32)
    Wk = np.asarray(Wk, np.float32)
    Wv = np.asarray(Wv, np.float32)
    Wc = np.asarray(Wc, np.float32)
    bv = np.asarray(bv, np.float32)
    bc = np.asarray(bc, np.float32)

    WkT = np.ascontiguousarray(Wk.transpose(1, 0, 2).reshape(D, H * E))
    WvT = np.ascontiguousarray(Wv.transpose(1, 0, 2).reshape(D, H * E))
    bc_eff = (
        np.asarray(bc, np.float64)
        + np.asarray(bv, np.float64).reshape(H * E) @ np.asarray(Wc, np.float64)
    ).astype(np.float32)
    bcT = np.ascontiguousarray(bc_eff.reshape(E, 1))
    WcC = np.ascontiguousarray(Wc)

    in_maps = []
    for c in range(N_CORES):
        in_maps.append(
            {
                "statesT": np.ascontiguousarray(states[c].T),  # [D, SK]
                "queryT": np.ascontiguousarray(query[c].T),    # [E, SQ]
                "WkT": WkT,
                "WvT": WvT,
                "Wc": WcC,
                "bcT": bcT,
                "onesSQ": _ONES_SQ,
            }
        )
    return in_maps


def get_nc(mm_dtype="f32r", repeat=1):
    key = (mm_dtype, repeat)
    nc = _COMPILED.get(key)
    if nc is None:
        nc = build_nc(mm_dtype, repeat=repeat)
        _COMPILED[key] = nc
    return nc


def kernel(query, states, Wk, bk, Wv, bv, Wc, bc):
    nc = get_nc()
    in_maps = _prep_inputs(query, states, Wk, bk, Wv, bv, Wc, bc)
    res = bass_utils.run_bass_kernel_spmd(nc, in_maps, list(range(N_CORES)))
    return np.stack([res.results[c]["out"] for c in range(N_CORES)], axis=0)


# revision 4
# speedup vs baseline: 1.7933x; 1.0129x over previous
"""Multi-head attention (lazy K/V projections) Trainium2 Bass kernel.

Problem: nn_MultiHeadAttention_54520314856024
  B=8, SQ=SK=1024, D=1024, E=128, H=32
  keys  = einsum('bsd,hde->hbse', states, Wk) + bk
  vals  = einsum('bsd,hde->hbse', states, Wv) + bv
  attn  = softmax(einsum('bqe,hbke->hbqk', query, keys) / sqrt(E))
  ctx   = einsum('hbqk,hbke->hbqe', attn, vals) -> concat heads -> @ Wc + bc

Sharding: batch-parallel, one batch element per NeuronCore (8 cores).

Design notes:
  - bk dropped on device: softmax over k is invariant to the per-q additive
    shift (bk . q), so the keys bias cancels exactly.
  - bv folded into bc on host: sum_k attn = 1 implies ctx = ctx0 + bv, so
    out = ctx0 @ Wc + (bc + bv.flatten() @ Wc). Removes all vals-bias work.
  - scores/keys/vals/transposes share one 3-buffer single-bank PSUM pool
    (their phases are disjoint); denominator + final-projection share a
    2-buffer pool; ctx accumulators a 2-buffer pool. 7 PSUM banks total.
  - per head, the kt loop emits scores+exp for kt and denominator/ctx
    matmuls for kt-1 (software pipelining) so PE never waits on ACT's exp.
  - the final projection of head h is deferred until after head h+1's keys
    so DVE recip/mul latency hides under PE keys matmuls.
"""

import sys

for _p in ("/opt/trn_rl_repo",):
    if _p not in sys.path:
        sys.path.insert(0, _p)

import numpy as np

import concourse.bass as bass
import concourse.mybir as mybir
import concourse.tile as tile
from concourse import bacc, bass_utils
from concourse.masks import make_identity

B, SQ, SK = 8, 1024, 1024
D, E, H = 1024, 128, 32
P = 128          # partition width
DCH = D // P     # 8 d-chunks
KT = SK // P     # 8 k-tiles
G = 4            # heads per vals-group
NG = H // G      # 8 groups
NHALF = 512      # matmul moving-dim chunk (fp32 max)
SCALE = 1.0 / float(np.sqrt(E))

F32 = mybir.dt.float32
F32R = mybir.dt.float32r

N_CORES = 8

_COMPILED = {}
_ONES_SQ = np.ones((P, P), np.float32)


def build_nc(mm_dtype="f32r", repeat=1):
    """Build the single-core Bass program (SPMD across 8 cores).

    repeat > 1 re-emits the whole computation that many times (identical
    work each pass) for launch-overhead-amortized timing; the final DRAM
    output is written by every pass (all identical).
    """
    MT = F32R if mm_dtype == "f32r" else F32

    nc = bacc.Bacc("TRN2", target_bir_lowering=False, debug=False)

    statesT = nc.dram_tensor("statesT", [D, SK], MT, kind="ExternalInput").ap()
    queryT = nc.dram_tensor("queryT", [E, SQ], MT, kind="ExternalInput").ap()
    WkT = nc.dram_tensor("WkT", [D, H * E], MT, kind="ExternalInput").ap()
    WvT = nc.dram_tensor("WvT", [D, H * E], MT, kind="ExternalInput").ap()
    Wc = nc.dram_tensor("Wc", [H * E, E], MT, kind="ExternalInput").ap()
    bcT = nc.dram_tensor("bcT", [E, 1], F32, kind="ExternalInput").ap()
    onesSQ = nc.dram_tensor("onesSQ", [P, P], MT, kind="ExternalInput").ap()
    out = nc.dram_tensor("out", [SQ, E], F32, kind="ExternalOutput").ap()

    Wc3 = Wc.rearrange("(h e) f -> h e f", e=P)

    from contextlib import ExitStack

    with tile.TileContext(nc) as tc, ExitStack() as es:
        constp = es.enter_context(tc.tile_pool(name="const", bufs=1))
        statesp = es.enter_context(tc.tile_pool(name="states", bufs=DCH))
        queryp = es.enter_context(tc.tile_pool(name="query", bufs=1))
        wkp = es.enter_context(tc.tile_pool(name="wk", bufs=10))
        wvp = es.enter_context(tc.tile_pool(name="wv", bufs=10))
        wcp = es.enter_context(tc.tile_pool(name="wc", bufs=4))
        keysp = es.enter_context(tc.tile_pool(name="keys", bufs=2))
        expp = es.enter_context(tc.tile_pool(name="exps", bufs=8))
        valsp = es.enter_context(tc.tile_pool(name="vals", bufs=12))
        recipp = es.enter_context(tc.tile_pool(name="recip", bufs=2))
        ctxp = es.enter_context(tc.tile_pool(name="ctx", bufs=2))
        finalp = es.enter_context(tc.tile_pool(name="final", bufs=1))
        outp = es.enter_context(tc.tile_pool(name="outs", bufs=4))
        ps_main = es.enter_context(tc.tile_pool(name="ps_main", bufs=4, space="PSUM"))
        ps_denom = es.enter_context(tc.tile_pool(name="ps_denom", bufs=2, space="PSUM"))
        ps_ctx = es.enter_context(tc.tile_pool(name="ps_ctx", bufs=2, space="PSUM"))

        # ---- constants ----
        ones_sq = constp.tile([P, P], MT)
        nc.sync.dma_start(ones_sq[:], onesSQ[:])
        ident = constp.tile([P, P], F32)
        make_identity(nc, ident[:])
        bc_t = constp.tile([E, 1], F32)
        nc.sync.dma_start(bc_t[:], bcT[:])

        # ---- resident activations ----
        st = []
        for d in range(DCH):
            st_t = statesp.tile([P, SK], MT, name="st_t")
            nc.sync.dma_start(st_t[:], statesT[d * P : (d + 1) * P, :])
            st.append(st_t)
        q_t = queryp.tile([E, SQ], MT)
        nc.sync.dma_start(q_t[:], queryT[:])

        final_t = finalp.tile([E, SQ], F32)

        def emit_final(rep, h, ctx_sb):
            """Final projection contribution of head h into final_t."""
            wc_t = wcp.tile([P, P], MT, name="wc_t")
            nc.sync.dma_start(wc_t[:], Wc3[h])
            for qh in range(2):
                pf = ps_denom.tile(
                    [P, NHALF], F32, tag="denom", name="pf"
                )
                nc.tensor.matmul(
                    pf[:],
                    (wc_t[:]),
                    (ctx_sb[:, qh * NHALF : (qh + 1) * NHALF]),
                    start=True,
                    stop=True,
                )
                if h == 0:
                    nc.vector.tensor_scalar(
                        final_t[:, qh * NHALF : (qh + 1) * NHALF],
                        pf[:],
                        bc_t[:],
                        None,
                        op0=mybir.AluOpType.add,
                    )
                else:
                    nc.vector.tensor_add(
                        final_t[:, qh * NHALF : (qh + 1) * NHALF],
                        final_t[:, qh * NHALF : (qh + 1) * NHALF],
                        pf[:],
                    )

        pending_final = None

        for rep in range(repeat):
            for g in range(NG):
                # ---- vals for this head-group: vals[k, (g4,e)] ----
                wv_tiles = []
                for d in range(DCH):
                    wv_t = wvp.tile([P, G * E], MT, name="wv_t")
                    nc.sync.dma_start(
                        wv_t[:],
                        WvT[d * P : (d + 1) * P, g * G * E : (g + 1) * G * E],
                    )
                    wv_tiles.append(wv_t)
                wk_tiles = []
                for d in range(DCH):
                    wk_t = wkp.tile([P, G * E], MT, name="wk_t")
                    nc.sync.dma_start(
                        wk_t[:],
                        WkT[d * P : (d + 1) * P, g * G * E : (g + 1) * G * E],
                    )
                    wk_tiles.append(wk_t)

                vals_tiles = []
                for kt in range(KT):
                    pv = ps_main.tile(
                        [P, G * E], F32, tag="m", name="pv"
                    )
                    for d in range(DCH):
                        nc.tensor.matmul(
                            pv[:],
                            (st[d][:, kt * P : (kt + 1) * P]),
                            (wv_tiles[d][:]),
                            start=(d == 0),
                            stop=(d == DCH - 1),
                        )
                    v_sb = valsp.tile([P, G * E], MT, name="v_sb")
                    nc.vector.tensor_copy(v_sb[:], pv[:])
                    vals_tiles.append(v_sb)

                for hg in range(G):
                    h = g * G + hg
                    # ---- keysT: [E, SK] (no bias: softmax-shift invariant) ----
                    keys_sb = keysp.tile([E, SK], MT, name="keys_sb")
                    for half in range(2):
                        pk = ps_main.tile(
                            [P, NHALF], F32, tag="m", name="pk"
                        )
                        for d in range(DCH):
                            nc.tensor.matmul(
                                pk[:],
                                (wk_tiles[d][:, hg * E : (hg + 1) * E]),
                                (st[d][:, half * NHALF : (half + 1) * NHALF]),
                                start=(d == 0),
                                stop=(d == DCH - 1),
                            )
                        nc.vector.tensor_copy(
                            keys_sb[:, half * NHALF : (half + 1) * NHALF], pk[:]
                        )

                    # previous head's final projection: PE runs it after keys,
                    # by which time DVE recip/mul of head h-1 has completed.
                    if pending_final is not None:
                        emit_final(*pending_final)
                        pending_final = None

                    # ---- scores -> exp -> denom/ctx accumulate, per (kt, qh) ----
                    pd = [
                        ps_denom.tile(
                            [P, NHALF], F32, tag="denom", name="pd"
                        )
                        for i in range(2)
                    ]
                    pc = [
                        ps_ctx.tile(
                            [P, NHALF], F32, tag="ctx", name="pc"
                        )
                        for i in range(2)
                    ]
                    # Software-pipelined: emit kt's scores+exp, then kt-1's
                    # denom/ctx (so PE never waits on the just-issued exp).
                    pending = None  # ([ex_qh0, ex_qh1], kt)
                    for kt in range(KT):
                        exs = []
                        for qh in range(2):
                            ps = ps_main.tile(
                                [P, NHALF], F32, tag="m",
                                name="ps",
                            )
                            nc.tensor.matmul(
                                ps[:],
                                (keys_sb[:, kt * P : (kt + 1) * P]),
                                (q_t[:, qh * NHALF : (qh + 1) * NHALF]),
                                start=True,
                                stop=True,
                            )
                            ex = expp.tile(
                                [P, NHALF], MT, name="ex"
                            )
                            nc.scalar.activation(
                                ex[:], ps[:], mybir.ActivationFunctionType.Exp,
                                scale=SCALE,
                            )
                            exs.append(ex)
                        if pending is not None:
                            pexs, pkt = pending
                            for qh in range(2):
                                nc.tensor.matmul(
                                    pd[qh][:],
                                    (ones_sq[:]),
                                    (pexs[qh][:]),
                                    start=(pkt == 0),
                                    stop=False,
                                )
                                nc.tensor.matmul(
                                    pc[qh][:],
                                    (vals_tiles[pkt][:, hg * E : (hg + 1) * E]),
                                    (pexs[qh][:]),
                                    start=(pkt == 0),
                                    stop=False,
                                )
                        pending = (exs, kt)
                    pexs, pkt = pending
                    for qh in range(2):
                        nc.tensor.matmul(
                            pd[qh][:],
                            (ones_sq[:]),
                            (pexs[qh][:]),
                            start=False,
                            stop=True,
                        )
                        nc.tensor.matmul(
                            pc[qh][:],
                            (vals_tiles[pkt][:, hg * E : (hg + 1) * E]),
                            (pexs[qh][:]),
                            start=False,
                            stop=True,
                        )

                    # ---- normalize: ctx_sb = pc / denom ----
                    ctx_sb = ctxp.tile([E, SQ], MT, name="ctx_sb")
                    for qh in range(2):
                        rec = recipp.tile(
                            [P, NHALF], F32, name="rec"
                        )
                        nc.vector.reciprocal_approx_fast(out=rec[:], in_=pd[qh][:])
                        nc.vector.tensor_mul(
                            ctx_sb[:, qh * NHALF : (qh + 1) * NHALF],
                            pc[qh][:],
                            rec[:],
                        )

                    # defer the final projection until after the next head's keys
                    pending_final = (rep, h, ctx_sb)

            emit_final(*pending_final)
            pending_final = None

            # ---- transpose finalT -> out [SQ, E] ----
            for qt in range(KT):
                pt = ps_main.tile([P, P], F32, tag="m", name="pt")
                nc.tensor.transpose(
                    pt[:], final_t[:, qt * P : (qt + 1) * P], ident[:]
                )
                o_sb = outp.tile([P, E], F32, name="o_sb")
                nc.vector.tensor_copy(o_sb[:], pt[:])
                nc.sync.dma_start(out[qt * P : (qt + 1) * P, :], o_sb[:])

    nc.compile()
    return nc


def _prep_inputs(query, states, Wk, bk, Wv, bv, Wc, bc):
    """Host-side sharding: per-core input maps (core c == batch element c).

    bk is dropped (softmax shift invariance); bv is folded into bc:
    out = ctx0 @ Wc + (bc + bv.flatten() @ Wc).
    """
    query = np.asarray(query, np.float32)
    states = np.asarray(states, np.float32)
    Wk = np.asarray(Wk, np.float32)
    Wv = np.asarray(Wv, np.float32)
    Wc = np.asarray(Wc, np.float32)
    bv = np.asarray(bv, np.float32)
    bc = np.asarray(bc, np.float32)

    WkT = np.ascontiguousarray(Wk.transpose(1, 0, 2).reshape(D, H * E))
    WvT = np.ascontiguousarray(Wv.transpose(1, 0, 2).reshape(D, H * E))
    bc_eff = (
        np.asarray(bc, np.float64)
        + np.asarray(bv, np.float64).reshape(H * E) @ np.asarray(Wc, np.float64)
    ).astype(np.float32)
    bcT = np.ascontiguousarray(bc_eff.reshape(E, 1))
    WcC = np.ascontiguousarray(Wc)

    in_maps = []
    for c in range(N_CORES):
        in_maps.append(
            {
                "statesT": np.ascontiguousarray(states[c].T),  # [D, SK]
                "queryT": np.ascontiguousarray(query[c].T),    # [E, SQ]
                "WkT": WkT,
                "WvT": WvT,
                "Wc": WcC,
                "bcT": bcT,
                "onesSQ": _ONES_SQ,
            }
        )
    return in_maps


def get_nc(mm_dtype="f32r", repeat=1):
    key = (mm_dtype, repeat)
    nc = _COMPILED.get(key)
    if nc is None:
        nc = build_nc(mm_dtype, repeat=repeat)
        _COMPILED[key] = nc
    return nc


def kernel(query, states, Wk, bk, Wv, bv, Wc, bc):
    nc = get_nc()
    in_maps = _prep_inputs(query, states, Wk, bk, Wv, bv, Wc, bc)
    res = bass_utils.run_bass_kernel_spmd(nc, in_maps, list(range(N_CORES)))
    return np.stack([res.results[c]["out"] for c in range(N_CORES)], axis=0)
